# revision 25
# baseline (speedup 1.0000x reference)
"""MoD Infini-Transformer Trainium2 kernel (self-contained).

Shards across 8 NeuronCores: core c = (b, half) with b = c//2, half = c%2.
Each core receives its batch row's sequence in LOCAL order
[own half | other half], so one SPMD program serves all cores; the
scan-order dependence of the compressive memory (which segment deltas
precede each own segment) is supplied as a tiny per-core prefix-selector
input W — the Infini-attention memory update is order-free linear
accumulation, so prefix state = data-driven weighted sum of per-segment
deltas.

Per-core pipeline:
  B: stream x (64 tiles): routing scores for all rows (fp32) + fused
     LayerNorm + y write for the own half.
  C: per full-segment top-256 threshold via 40-step branchless bisection.
  D: mask -> rank (triangular-matmul column cumsum + segment offsets) ->
     compaction to sorted index lists via one-hot x iota matmuls.
  E: indirect-DMA gather of the 1024 selected rows; PE transpose.
  F: q/k (own), sk/v (all) projections in bf16.
  G: per-segment memory deltas (sk^T v, sum sk); W-weighted prefix states.
  H: per (own seg, head): softmax attention + compressive-memory
     attention, gate combine (f32r for the memory path).
  I: out-projection + MLP (bf16, weights streamed).
  J: h^T transpose, x+h, LayerNorm, indirect scatter of updated rows
     into y; mask scatter; scores output.
"""
import numpy as np
import ml_dtypes

import concourse.bass as bass
import concourse.tile as tile
from concourse import bacc, mybir
from concourse.masks import make_identity
from concourse import bass_utils

F32 = mybir.dt.float32
F32R = mybir.dt.float32r
BF16 = mybir.dt.bfloat16
I32 = mybir.dt.int32
AX = mybir.AxisListType
OP = mybir.AluOpType
AF = mybir.ActivationFunctionType

# problem constants
B, S, D = 4, 8192, 1024
H, DK, DV = 8, 64, 64
DH = 4096
FULL, SAMP = 2048, 8
SEG = FULL // SAMP            # 256
NSEG = 4                      # full segments per row
SEL = NSEG * SEG              # 1024 selected tokens per row
HALF = S // 2                 # 4096
NT_X = S // 128               # 64 streaming tiles
NT_OWN = HALF // 128          # 32
NT_SEL = SEL // 128           # 8 selected-token chunks
OWN_TOK = 512                 # own selected tokens
NITER = 40                    # bisection iterations
EPS = 1e-5
N_CORES = 8


def build_kernel(nc):
    # ---------------- DRAM I/O ----------------
    x = nc.dram_tensor("x", (S, D), F32, kind="ExternalInput")
    wq = nc.dram_tensor("wq", (D, H * DK), BF16, kind="ExternalInput")
    wk = nc.dram_tensor("wk", (D, H * DK), BF16, kind="ExternalInput")
    wv = nc.dram_tensor("wv", (D, H * DV), BF16, kind="ExternalInput")
    wo = nc.dram_tensor("wo", (H * DV, D), BF16, kind="ExternalInput")
    w1 = nc.dram_tensor("w1", (D, DH), BF16, kind="ExternalInput")
    w2 = nc.dram_tensor("w2", (DH, D), BF16, kind="ExternalInput")
    wsamp = nc.dram_tensor("wsamp", (D, 1), F32, kind="ExternalInput")
    bsamp = nc.dram_tensor("bsamp", (1, 1), F32, kind="ExternalInput")
    betas = nc.dram_tensor("betas", (H * DV, 1), F32, kind="ExternalInput")
    b1 = nc.dram_tensor("b1", (DH, 1), F32, kind="ExternalInput")
    b2 = nc.dram_tensor("b2", (D, 1), F32, kind="ExternalInput")
    gamma = nc.dram_tensor("gamma", (D, 1), F32, kind="ExternalInput")
    lnbeta = nc.dram_tensor("lnbeta", (D, 1), F32, kind="ExternalInput")
    wpre = nc.dram_tensor("wpre", (1, 8), F32, kind="ExternalInput")

    y = nc.dram_tensor("y", (HALF, D), F32, kind="ExternalOutput")
    sc_out = nc.dram_tensor("sc_out", (HALF, 1), F32, kind="ExternalOutput")
    mk_out = nc.dram_tensor("mk_out", (HALF, 1), F32, kind="ExternalOutput")

    with tile.TileContext(nc) as tc:
        _body(tc, dict(x=x, wq=wq, wk=wk, wv=wv, wo=wo, w1=w1, w2=w2,
                       wsamp=wsamp, bsamp=bsamp, betas=betas, b1=b1, b2=b2,
                       gamma=gamma, lnbeta=lnbeta, wpre=wpre,
                       y=y, sc_out=sc_out, mk_out=mk_out))
    return nc


def _body(tc, t):
    from contextlib import ExitStack
    nc = tc.nc
    x, y = t["x"], t["y"]

    es = ExitStack()
    cpool = es.enter_context(tc.tile_pool(name="const", bufs=1))
    spool = es.enter_context(tc.tile_pool(name="sbuf", bufs=1))
    work = es.enter_context(tc.tile_pool(name="work", bufs=2))
    wpool = es.enter_context(tc.tile_pool(name="wstream", bufs=1))
    xpool = es.enter_context(tc.tile_pool(name="xstream", bufs=3))
    ypool = es.enter_context(tc.tile_pool(name="ystream", bufs=2))
    poolB = tc.tile_pool(name="poolB", bufs=1)
    poolB_p = poolB.__enter__()
    ps1 = tc.tile_pool(name="ps1", bufs=2, space="PSUM")
    ps1_pool = ps1.__enter__()

    def p1(shape, dtype=F32, tag="b1", bufs=None):
        return ps1_pool.tile(shape, dtype, space="PSUM", tag=tag,
                             name=f"ps_{tag}", bufs=(4 if tag == "b1" else bufs))

    # ---------------- constants ----------------
    ident = cpool.tile([128, 128], F32)
    make_identity(nc, ident[:])

    ones_col = cpool.tile([1, 128], F32)       # lhsT for partition-broadcast
    nc.vector.memset(ones_col[:], 1.0)
    ones128 = cpool.tile([128, 1], F32)        # lhsT for partition-sum
    nc.vector.memset(ones128[:], 1.0)
    ones_bf = cpool.tile([128, 1], BF16)
    nc.vector.memset(ones_bf[:], 1.0)
    eps_t = cpool.tile([128, 1], F32)
    nc.vector.memset(eps_t[:], EPS)

    # tri[k, m] = 1 if k <= m: strict-upper via affine_select, + identity
    tri = cpool.tile([128, 128], F32)
    nc.gpsimd.memset(tri[:], 0.0)
    nc.gpsimd.affine_select(out=tri[:], in_=tri[:], compare_op=OP.is_ge,
                            fill=1.0, base=0, pattern=[[-1, 128]],
                            channel_multiplier=1)
    nc.vector.tensor_add(tri[:], tri[:], ident[:])

    # iota_r[p, r] = r
    iota_r_i = work.tile([128, SEG], I32, tag="iota_r_i", bufs=1)
    nc.gpsimd.iota(iota_r_i[:], pattern=[[1, SEG]], base=0, channel_multiplier=0)
    iota_r = cpool.tile([128, SEG], F32)
    nc.vector.tensor_copy(iota_r[:], iota_r_i[:])

    # growp[p] = p; growc[p, c] = c   (bf16-exact values)
    gp_i = work.tile([128, 1], I32, tag="gp_i", bufs=1)
    nc.gpsimd.iota(gp_i[:], pattern=[[1, 1]], base=0, channel_multiplier=1)
    growp = cpool.tile([128, 1], BF16)
    nc.vector.tensor_copy(growp[:], gp_i[:])
    gc_i = work.tile([128, 16], I32, tag="gc_i", bufs=1)
    nc.gpsimd.iota(gc_i[:], pattern=[[1, 16]], base=0, channel_multiplier=0)
    growc = cpool.tile([128, 16], BF16)
    nc.vector.tensor_copy(growc[:], gc_i[:])

    # broadcasts of small params: DRAM (n,1) -> [1,n] -> psum [128,n] -> sbuf
    def bcast_cols(src_dram, n, name, pool=None):
        row = poolB_p.tile([1, n], F32, tag="bc_row", bufs=1, name=f"row_{name}")
        nc.sync.dma_start(out=row[:], in_=src_dram.ap().rearrange("d o -> o d"))
        out = (pool or cpool).tile([128, n], F32, tag=name, name=f"bc_{name}")
        for j in range(0, n, 512):
            w = min(512, n - j)
            pt = p1([128, w], F32, tag="b2")
            nc.tensor.matmul(pt[:], ones_col[:], row[:, j:j + w],
                             start=True, stop=True)
            nc.vector.tensor_copy(out[:, j:j + w], pt[:])
        return out

    w_rep = bcast_cols(t["wsamp"], D, "w_rep", pool=poolB_p)
    gamma_rep = bcast_cols(t["gamma"], D, "gamma_rep")
    beta_rep = bcast_cols(t["lnbeta"], D, "beta_rep")
    bs_rep = bcast_cols(t["bsamp"], 1, "bs_rep")

    wrow = work.tile([1, 8], F32, tag="wpre_row")
    nc.sync.dma_start(out=wrow[:], in_=t["wpre"][:, :])
    wb_ps = p1([128, 8], F32, tag="b1")
    nc.tensor.matmul(wb_ps[:], ones_col[:], wrow[:], start=True, stop=True)
    wb = cpool.tile([128, 8], F32)
    nc.vector.tensor_copy(wb[:], wb_ps[:])

    # betas -> [128, 4] (col k = hd chunk k), gate = sigmoid
    betas_sb2 = work.tile([64, H], F32, tag="betas_sb2")
    nc.sync.dma_start(out=betas_sb2[:],
                      in_=t["betas"].ap().rearrange("(c p) o -> p (c o)", p=64))
    gate_sb = cpool.tile([64, H], F32)
    nc.scalar.activation(gate_sb[:], betas_sb2[:], AF.Sigmoid)

    b1_sb = cpool.tile([128, 32], F32)
    nc.sync.dma_start(out=b1_sb[:],
                      in_=t["b1"].ap().rearrange("(m p) o -> p (m o)", p=128))
    b2_sb = cpool.tile([128, 8], F32)
    nc.sync.dma_start(out=b2_sb[:],
                      in_=t["b2"].ap().rearrange("(m p) o -> p (m o)", p=128))

    scores_sb = cpool.tile([128, NT_X], F32)   # scores_sb[p, i] = score(row 128i+p)

    # ---------------- phase B: stream x, scores + own-half LN ----------------
    def layer_norm(src_ap):
        """src [128,1024] -> normalized [128,1024] f32 tile (gamma/beta applied)."""
        sumx = work.tile([128, 1], F32, tag="ln_sum")
        nc.vector.tensor_reduce(sumx[:], src_ap, axis=AX.X, op=OP.add)
        sq = work.tile([128, 1024], F32, tag="ln_sq", bufs=1)
        sumsq = work.tile([128, 1], F32, tag="ln_sumsq")
        nc.vector.tensor_mul(sq[:], src_ap, src_ap)
        nc.vector.tensor_reduce(sumsq[:], sq[:], axis=AX.X, op=OP.add)
        mu = work.tile([128, 1], F32, tag="ln_mu")
        nc.vector.tensor_scalar_mul(mu[:], sumx[:], 1.0 / D)
        msq = work.tile([128, 1], F32, tag="ln_msq")
        nc.vector.scalar_tensor_tensor(
            out=msq[:], in0=mu[:], scalar=-1.0, in1=mu[:],
            op0=OP.mult, op1=OP.mult)
        var = work.tile([128, 1], F32, tag="ln_var")
        nc.vector.scalar_tensor_tensor(
            out=var[:], in0=sumsq[:], scalar=1.0 / D, in1=msq[:],
            op0=OP.mult, op1=OP.add)
        sd = work.tile([128, 1], F32, tag="ln_sd")
        nc.scalar.activation(sd[:], var[:], AF.Sqrt, bias=eps_t[:], scale=1.0)
        rstd = work.tile([128, 1], F32, tag="ln_rstd")
        nc.vector.reciprocal(rstd[:], sd[:])
        nmr = work.tile([128, 1], F32, tag="ln_nmr")
        nc.vector.scalar_tensor_tensor(
            out=nmr[:], in0=mu[:], scalar=-1.0, in1=rstd[:],
            op0=OP.mult, op1=OP.mult)
        yt = ypool.tile([128, 1024], F32, tag="ln_out")
        nc.scalar.activation(yt[:], src_ap, AF.Identity, bias=nmr[:], scale=rstd[:])
        nc.vector.tensor_mul(yt[:], yt[:], gamma_rep[:])
        nc.vector.tensor_add(yt[:], yt[:], beta_rep[:])
        return yt

    for i in range(NT_X):
        xt = xpool.tile([128, 1024], F32, tag="xin")
        nc.sync.dma_start(out=xt[:], in_=x[128 * i:128 * (i + 1), :])
        sc_scratch = poolB_p.tile([128, 1024], F32, tag="sc_scratch",
                                  name="sc_scratch")
        nc.vector.tensor_mul(sc_scratch[:], xt[:], w_rep[:])
        nc.vector.tensor_reduce(scores_sb[:, i:i + 1], sc_scratch[:],
                                axis=AX.X, op=OP.add)
        if i < NT_OWN:
            yt = layer_norm(xt[:])
            nc.sync.dma_start(out=y[128 * i:128 * (i + 1), :], in_=yt[:])

    poolB.__exit__(None, None, None)

    # ---------------- phase C: bisection thresholds ----------------
    lo = spool.tile([1, 4], F32, tag="lo")
    hi = spool.tile([1, 4], F32, tag="hi")
    mid = spool.tile([1, 4], F32, tag="mid")
    nc.vector.memset(lo[:], -16.0)
    nc.vector.memset(hi[:], 16.0)
    nc.vector.memset(mid[:], 0.0)
    for it in range(NITER):
        mid_ps = p1([128, 4], F32, tag="b1")
        nc.tensor.matmul(mid_ps[:], ones_col[:], mid[:], start=True, stop=True)
        cmp_t = work.tile([128, NT_X], F32, tag="cmp")
        for g in range(4):
            nc.vector.tensor_tensor(
                out=cmp_t[:, 16 * g:16 * (g + 1)],
                in0=scores_sb[:, 16 * g:16 * (g + 1)],
                in1=mid_ps[:, g:g + 1].to_broadcast([128, 16]),
                op=OP.is_ge)
        cnt_t = work.tile([128, 4], F32, tag="cnt")
        nc.vector.tensor_reduce(
            cnt_t[:], cmp_t[:].rearrange("p (g c) -> p g c", g=4),
            axis=AX.X, op=OP.add)
        cnt_ps = p1([1, 4], F32, tag="b1")
        nc.tensor.matmul(cnt_ps[:], ones128[:], cnt_t[:], start=True, stop=True)
        ge = work.tile([1, 4], F32, tag="bis_ge")
        nge = work.tile([1, 4], F32, tag="bis_nge")
        nc.vector.tensor_scalar(ge[:], cnt_ps[:], float(SEG), None, op0=OP.is_ge)
        nc.vector.tensor_scalar(nge[:], cnt_ps[:], float(SEG), None, op0=OP.is_lt)
        # lo += ge*(mid-lo); hi += nge*(mid-hi)
        d1 = work.tile([1, 4], F32, tag="bis_d1")
        nc.vector.tensor_sub(d1[:], mid[:], lo[:])
        nc.vector.tensor_mul(d1[:], d1[:], ge[:])
        nc.vector.tensor_add(lo[:], lo[:], d1[:])
        d2 = work.tile([1, 4], F32, tag="bis_d2")
        nc.vector.tensor_sub(d2[:], mid[:], hi[:])
        nc.vector.tensor_mul(d2[:], d2[:], nge[:])
        nc.vector.tensor_add(hi[:], hi[:], d2[:])
        tsum = work.tile([1, 4], F32, tag="bis_tsum")
        nc.vector.tensor_add(tsum[:], lo[:], hi[:])
        nc.vector.tensor_scalar_mul(mid[:], tsum[:], 0.5)

    # ---------------- phase D: mask, rank, compact ----------------
    thr_ps = p1([128, 4], F32, tag="b1")
    nc.tensor.matmul(thr_ps[:], ones_col[:], lo[:], start=True, stop=True)
    mask_t = spool.tile([128, NT_X], F32, tag="mask")
    for g in range(4):
        nc.vector.tensor_tensor(
            out=mask_t[:, 16 * g:16 * (g + 1)],
            in0=scores_sb[:, 16 * g:16 * (g + 1)],
            in1=thr_ps[:, g:g + 1].to_broadcast([128, 16]),
            op=OP.is_ge)
    csum_ps = p1([128, NT_X], F32, tag="b1")
    nc.tensor.matmul(csum_ps[:], tri[:], mask_t[:], start=True, stop=True)
    csum_sb = work.tile([128, NT_X], F32, tag="csum_sb")
    nc.vector.tensor_copy(csum_sb[:], csum_ps[:])
    tot_ps = p1([1, NT_X], F32, tag="b1")
    nc.tensor.matmul(tot_ps[:], ones128[:], mask_t[:], start=True, stop=True)
    tot = work.tile([1, NT_X], F32, tag="tot")
    nc.vector.tensor_copy(tot[:], tot_ps[:])
    incl = work.tile([1, NT_X], F32, tag="incl")
    zrow = work.tile([1, NT_X], F32, tag="zrow")
    nc.vector.memset(zrow[:], 0.0)
    for g in range(4):
        nc.vector.tensor_tensor_scan(
            out=incl[:, 16 * g:16 * (g + 1)],
            data0=tot[:, 16 * g:16 * (g + 1)],
            data1=zrow[:, 16 * g:16 * (g + 1)],
            initial=0.0, op0=OP.add, op1=OP.add)
    ex = work.tile([1, NT_X], F32, tag="ex")
    nc.vector.tensor_sub(ex[:], incl[:], tot[:])
    exb_ps = p1([128, NT_X], F32, tag="b1")
    nc.tensor.matmul(exb_ps[:], ones_col[:], ex[:], start=True, stop=True)
    rank_m = spool.tile([128, NT_X], F32, tag="rank_m")
    nc.vector.tensor_add(rank_m[:], csum_sb[:], exb_ps[:])
    nc.vector.tensor_mul(rank_m[:], rank_m[:], mask_t[:])
    nc.vector.tensor_scalar_add(rank_m[:], rank_m[:], -1.0)

    sel_f = spool.tile([128, NT_SEL], F32, tag="sel_f")
    sel_i = spool.tile([128, NT_SEL], I32, tag="sel_i")
    iota_bc = iota_r[:].rearrange("p r -> p () r").to_broadcast([128, 16, SEG])
    for g in range(4):
        E = work.tile([128, 16 * SEG], BF16, tag="Emat", bufs=1)
        nc.vector.tensor_tensor(
            out=E[:].rearrange("p (c r) -> p c r", c=16),
            in0=rank_m[:, 16 * g:16 * (g + 1)].rearrange(
                "p c -> p c ()").to_broadcast([128, 16, SEG]),
            in1=iota_bc,
            op=OP.is_equal)
        Ev = E[:].rearrange("p (c r) -> p c r", c=16)
        for rc in range(2):
            sel_pa = p1([128, 1], F32, tag="b1")
            for c in range(16):
                nc.tensor.matmul(
                    sel_pa[:], Ev[:, c, 128 * rc:128 * (rc + 1)],
                    growp[:, 0:1], start=(c == 0), stop=(c == 15))
            sel_pb = p1([128, 1], F32, tag="b1")
            for c in range(16):
                nc.tensor.matmul(
                    sel_pb[:], Ev[:, c, 128 * rc:128 * (rc + 1)],
                    growc[:, c:c + 1], start=(c == 0), stop=(c == 15))
            col = 2 * g + rc
            sel_sb = work.tile([128, 2], F32, tag="sel_sb")
            nc.vector.tensor_copy(sel_sb[:, 0:1], sel_pa[:])
            nc.vector.tensor_copy(sel_sb[:, 1:2], sel_pb[:])
            nc.vector.scalar_tensor_tensor(
                out=sel_f[:, col:col + 1], in0=sel_sb[:, 1:2], scalar=128.0,
                in1=sel_sb[:, 0:1], op0=OP.mult, op1=OP.add)
            nc.vector.tensor_scalar_add(
                sel_f[:, col:col + 1], sel_f[:, col:col + 1], float(2048 * g))
    nc.vector.tensor_copy(sel_i[:], sel_f[:])

    # ---------------- phase E: gather + transpose ----------------
    poolQ = tc.tile_pool(name="poolQ", bufs=1)
    poolQ_p = poolQ.__enter__()
    poolEF = tc.tile_pool(name="poolEF", bufs=1)
    poolEF_p = poolEF.__enter__()
    poolW = tc.tile_pool(name="poolW", bufs=1)
    poolW_p = poolW.__enter__()
    wq_sb = [poolW_p.tile([128, 512], BF16, tag=f"wq{d}", name=f"wq_sb{d}")
             for d in range(8)]
    wk_sb = [poolW_p.tile([128, 512], BF16, tag=f"wk{d}", name=f"wk_sb{d}")
             for d in range(8)]
    wv_sb = [poolW_p.tile([128, 512], BF16, tag=f"wv{d}", name=f"wv_sb{d}")
             for d in range(8)]
    for d in range(8):
        nc.sync.dma_start(out=wq_sb[d][:], in_=t["wq"][128 * d:128 * (d + 1), :])
        nc.sync.dma_start(out=wk_sb[d][:], in_=t["wk"][128 * d:128 * (d + 1), :])
        nc.sync.dma_start(out=wv_sb[d][:], in_=t["wv"][128 * d:128 * (d + 1), :])
    xT = [poolEF_p.tile([128, 1024], BF16, tag=f"xT{dd}", name=f"xT{dd}")
          for dd in range(8)]
    for ti in range(NT_SEL):
        xg_s = poolEF_p.tile([128, 1024], F32, tag="xg", bufs=2, name="xg_s")
        nc.gpsimd.indirect_dma_start(
            out=xg_s[:], out_offset=None, in_=x[:, :],
            in_offset=bass.IndirectOffsetOnAxis(ap=sel_i[:, ti:ti + 1], axis=0))
        for dd in range(8):
            px = p1([128, 128], F32, tag="b1")
            nc.tensor.transpose(px[:], xg_s[:, 128 * dd:128 * (dd + 1)], ident[:])
            nc.vector.tensor_copy(xT[dd][:, 128 * ti:128 * (ti + 1)], px[:])

    # ---------------- phase F: projections ----------------
    def elu1(psum_ap, out_tile, tag):
        """out = elu(psum)+1 = relu(psum) + exp(min(psum,0))."""
        m0 = work.tile(list(psum_ap.shape), F32, tag="elu_m0", bufs=1)
        nc.vector.tensor_scalar_min(m0[:], psum_ap, 0.0)
        e0 = work.tile(list(psum_ap.shape), F32, tag="elu_e0", bufs=1)
        nc.scalar.activation(e0[:], m0[:], AF.Exp)
        r0 = work.tile(list(psum_ap.shape), F32, tag="elu_r0", bufs=1)
        nc.scalar.activation(r0[:], psum_ap, AF.Relu)
        nc.vector.tensor_add(out_tile[:], e0[:], r0[:])

    sk_nat = [poolQ_p.tile([128, 512], BF16, tag=f"sk{ti}", name=f"sk{ti}") for ti in range(8)]
    v_nat = [poolQ_p.tile([128, 512], BF16, tag=f"v{ti}", name=f"v{ti}") for ti in range(8)]
    for ti in range(8):
        pk = p1([128, 512], F32, tag="b1")
        for dd in range(8):
            nc.tensor.matmul(pk[:], xT[dd][:, 128 * ti:128 * (ti + 1)],
                             wk_sb[dd][:], start=(dd == 0), stop=(dd == 7))
        elu1(pk[:], sk_nat[ti], "sk")
        pv = p1([128, 512], F32, tag="b1")
        for dd in range(8):
            nc.tensor.matmul(pv[:], xT[dd][:, 128 * ti:128 * (ti + 1)],
                             wv_sb[dd][:], start=(dd == 0), stop=(dd == 7))
        nc.vector.tensor_copy(v_nat[ti][:], pv[:])

    qT = [poolQ_p.tile([128, OWN_TOK], BF16, tag=f"qT{m}", name=f"qT{m}") for m in range(4)]
    sqT = [poolQ_p.tile([128, OWN_TOK], F32, tag=f"sqT{m}", name=f"sqT{m}") for m in range(4)]
    kT = [poolQ_p.tile([128, OWN_TOK], BF16, tag=f"kT{m}", name=f"kTt{m}") for m in range(4)]
    for m in range(4):
        pq = p1([128, OWN_TOK], F32, tag="b1")
        for dd in range(8):
            nc.tensor.matmul(pq[:], wq_sb[dd][:, 128 * m:128 * (m + 1)],
                             xT[dd][:, 0:OWN_TOK], start=(dd == 0), stop=(dd == 7))
        nc.vector.tensor_copy(qT[m][:], pq[:])
        elu1(pq[:], sqT[m], "sq")
        pkt = p1([128, OWN_TOK], F32, tag="b1")
        for dd in range(8):
            nc.tensor.matmul(pkt[:], wk_sb[dd][:, 128 * m:128 * (m + 1)],
                             xT[dd][:, 0:OWN_TOK], start=(dd == 0), stop=(dd == 7))
        nc.vector.tensor_copy(kT[m][:], pkt[:])

    poolW.__exit__(None, None, None)
    poolEF.__exit__(None, None, None)

    # per-head base-0 handles: even heads = slices, odd heads = copies
    qTo = [poolQ_p.tile([64, OWN_TOK], BF16, tag=f"qTo{c}", name=f"qTo{c}")
           for c in range(4)]
    kTo = [poolQ_p.tile([64, OWN_TOK], BF16, tag=f"kTo{c}", name=f"kTo{c}")
           for c in range(4)]
    sqTo = [poolQ_p.tile([64, OWN_TOK], F32, tag=f"sqTo{c}", name=f"sqTo{c}")
            for c in range(4)]
    for c in range(4):
        nc.vector.tensor_copy(qTo[c][:], qT[c][64:128, :])
        nc.vector.tensor_copy(kTo[c][:], kT[c][64:128, :])
        nc.vector.tensor_copy(sqTo[c][:], sqT[c][64:128, :])
    qT_h = [qT[h // 2][0:64, :] if h % 2 == 0 else qTo[h // 2][:]
            for h in range(H)]
    kT_h = [kT[h // 2][0:64, :] if h % 2 == 0 else kTo[h // 2][:]
            for h in range(H)]
    sqT_h = [sqT[h // 2][0:64, :] if h % 2 == 0 else sqTo[h // 2][:]
             for h in range(H)]

    # ---------------- phase G: deltas + prefix states ----------------
    # deltas[g][hc]: [128, 65] f32 — head 2hc on partitions 0:64, head 2hc+1
    # on 64:128; cols 0:64 = sk^T v, col 64 = sum sk
    deltas = [[poolQ_p.tile([128, 65], F32, tag=f"del{g}_{hc}", name=f"del{g}_{hc}")
               for hc in range(4)] for g in range(4)]
    for g in range(4):
        for h in range(H):
            hc, hp = h // 2, (h % 2) * 64
            pd = p1([64, 128], F32, tag="b1")
            for i2 in range(2):
                ti = 2 * g + i2
                nc.tensor.matmul(pd[:, 0:64],
                                 sk_nat[ti][:, 64 * h:64 * (h + 1)],
                                 v_nat[ti][:, 64 * h:64 * (h + 1)],
                                 start=(i2 == 0), stop=(i2 == 1))
            for i2 in range(2):
                ti = 2 * g + i2
                nc.tensor.matmul(pd[:, 64:65],
                                 sk_nat[ti][:, 64 * h:64 * (h + 1)],
                                 ones_bf[:],
                                 start=(i2 == 0), stop=(i2 == 1))
            nc.vector.tensor_copy(deltas[g][hc][hp:hp + 64, :], pd[:, 0:65])

    # M_own[j][hc] (f32r): prefix mem/z for own segment j, head pair hc
    M_own = [[poolQ_p.tile([128, 65], F32, tag=f"M{j}_{hc}", name=f"M{j}_{hc}")
              for hc in range(4)] for j in range(2)]
    for j in range(2):
        for hc in range(4):
            acc = M_own[j][hc]
            for g in range(4):
                nc.vector.scalar_tensor_tensor(
                    out=acc[:], in0=deltas[g][hc][:],
                    scalar=wb[:, 4 * j + g:4 * j + g + 1],
                    in1=(deltas[g][hc][:] if g == 0 else acc[:]),
                    op0=OP.mult, op1=(OP.bypass if g == 0 else OP.add))
            nc.vector.tensor_scalar_add(acc[:, 64:65], acc[:, 64:65], 1.0 / DK)

    Mo = [[poolQ_p.tile([64, 65], F32, tag=f"Mo{j}_{c}", name=f"Mo{j}_{c}")
           for c in range(4)] for j in range(2)]
    for j in range(2):
        for c in range(4):
            nc.vector.tensor_copy(Mo[j][c][:], M_own[j][c][64:128, :])
    M_h = [[(M_own[j][h // 2][0:64, :] if h % 2 == 0 else Mo[j][h // 2][:])
            for h in range(H)] for j in range(2)]

    # ---------------- phase H: attention over own segments ----------------
    att_h = [spool.tile([64, OWN_TOK], BF16, tag=f"atth{h}", name=f"atth{h}")
             for h in range(H)]
    for j in range(2):
        for hg in range(2):        # head groups of 4
            for sc in range(2):    # s-chunks of 128
                ps_s = p1([128, 1024], F32, tag="b2")
                for h2 in range(4):
                    h = 4 * hg + h2
                    hc, hp = h // 2, (h % 2) * 64
                    nc.tensor.matmul(
                        ps_s[:, 256 * h2:256 * (h2 + 1)],
                        qT_h[h][:, 256 * j + 128 * sc:256 * j + 128 * (sc + 1)],
                        kT_h[h][:, 256 * j:256 * (j + 1)],
                        start=True, stop=True)
                pmax = work.tile([128, 4], F32, tag="pmax")
                nc.vector.tensor_reduce(
                    pmax[:], ps_s[:].rearrange("p (h r) -> p h r", h=4),
                    axis=AX.X, op=OP.max)
                nm8 = work.tile([128, 4], F32, tag="nm8")
                nc.vector.tensor_scalar_mul(nm8[:], pmax[:], -0.125)
                p_sb = work.tile([128, 1024], F32, tag="p_sb", bufs=1)
                se = work.tile([128, 4], F32, tag="sumexp")
                for h2 in range(4):
                    nc.scalar.activation(
                        p_sb[:, 256 * h2:256 * (h2 + 1)],
                        ps_s[:, 256 * h2:256 * (h2 + 1)],
                        AF.Exp, bias=nm8[:, h2:h2 + 1], scale=0.125,
                        accum_out=se[:, h2:h2 + 1])
                rp = work.tile([128, 4], F32, tag="rp")
                nc.vector.reciprocal(rp[:], se[:])
                pn = work.tile([128, 1024], F32, tag="pn", bufs=1)
                for h2 in range(4):
                    nc.vector.tensor_scalar_mul(
                        pn[:, 256 * h2:256 * (h2 + 1)],
                        p_sb[:, 256 * h2:256 * (h2 + 1)], rp[:, h2:h2 + 1])
                for h2 in range(4):
                    h = 4 * hg + h2
                    hc, hp = h // 2, (h % 2) * 64
                    ppt = p1([128, 256], F32, tag="b1")
                    for tc2 in range(2):
                        nc.tensor.transpose(
                            ppt[:, 128 * tc2:128 * (tc2 + 1)],
                            pn[:, 256 * h2 + 128 * tc2:256 * h2 + 128 * (tc2 + 1)],
                            ident[:])
                    pT = work.tile([128, 256], BF16, tag="pT")
                    nc.vector.tensor_copy(pT[:], ppt[:])
                    scol = 256 * j + 128 * sc
                    # base-0 psum tile: cols 0:128 att_dotT, 128:256 att_memT,
                    # 256:384 broadcast of 1/denom
                    pk = p1([64, 384], F32, tag="b1")
                    for tc2 in range(2):
                        nc.tensor.matmul(
                            pk[:, 0:128], v_nat[2 * j + tc2][:, 64 * h:64 * (h + 1)],
                            pT[:, 128 * tc2:128 * (tc2 + 1)],
                            start=(tc2 == 0), stop=(tc2 == 1))
                    nc.tensor.matmul(pk[:, 128:256], M_h[j][h][:, 0:64],
                                     sqT_h[h][:, scol:scol + 128],
                                     start=True, stop=True)
                    pden = p1([1, 128], F32, tag="b1")
                    nc.tensor.matmul(pden[:], M_h[j][h][:, 64:65],
                                     sqT_h[h][:, scol:scol + 128],
                                     start=True, stop=True)
                    rd = work.tile([1, 128], F32, tag="rd")
                    nc.vector.reciprocal(rd[:], pden[:])
                    nc.tensor.matmul(pk[:, 256:384], ones_col[:, 0:64], rd[:],
                                     start=True, stop=True)
                    r_sb = work.tile([64, 128], F32, tag="r_sb")
                    nc.vector.tensor_copy(r_sb[:], pk[:, 256:384])
                    t1 = work.tile([64, 128], F32, tag="att_t1")
                    nc.vector.tensor_mul(t1[:], pk[:, 128:256], r_sb[:])
                    t2 = work.tile([64, 128], F32, tag="att_t2")
                    nc.vector.tensor_sub(t2[:], t1[:], pk[:, 0:128])
                    nc.vector.scalar_tensor_tensor(
                        out=att_h[h][:, scol:scol + 128],
                        in0=t2[:], scalar=gate_sb[:, h:h + 1],
                        in1=pk[:, 0:128], op0=OP.mult, op1=OP.add)

    poolQ.__exit__(None, None, None)

    # ---------------- phase I: out-proj + MLP ----------------
    poolL = tc.tile_pool(name="poolL", bufs=1)
    poolL_p = poolL.__enter__()
    wo_sb = [poolL_p.tile([64, 1024], BF16, tag=f"wo{k}", name=f"wo_sb{k}")
             for k in range(H)]
    for k in range(H):
        nc.sync.dma_start(out=wo_sb[k][:], in_=t["wo"][64 * k:64 * (k + 1), :])
    out_aT = [spool.tile([128, OWN_TOK], BF16, tag=f"oaT{m}", name=f"oaT{m}") for m in range(8)]
    for m in range(8):
        po = p1([128, OWN_TOK], F32, tag="b1")
        for kc in range(H):
            nc.tensor.matmul(po[:], wo_sb[kc][:, 128 * m:128 * (m + 1)],
                             att_h[kc][:], start=(kc == 0), stop=(kc == 7))
        nc.vector.tensor_copy(out_aT[m][:], po[:])

    h1T = [poolL_p.tile([128, OWN_TOK], BF16, tag=f"h1T{m}", name=f"h1T{m}")
           for m in range(32)]
    for ms in range(8):            # w1 slabs of 4 m-chunks
        w1s = [wpool.tile([128, 512], BF16, tag=f"w1s{dd}", name=f"w1s{dd}") for dd in range(8)]
        for dd in range(8):
            nc.sync.dma_start(
                out=w1s[dd][:],
                in_=t["w1"][128 * dd:128 * (dd + 1), 512 * ms:512 * (ms + 1)])
        for mi in range(4):
            m = 4 * ms + mi
            ph = p1([128, OWN_TOK], F32, tag="b1")
            for dd in range(8):
                nc.tensor.matmul(ph[:], w1s[dd][:, 128 * mi:128 * (mi + 1)],
                                 out_aT[dd][:], start=(dd == 0), stop=(dd == 7))
            nc.scalar.activation(h1T[m][:], ph[:], AF.Relu,
                                 bias=b1_sb[:, m:m + 1], scale=1.0)

    ps1.__exit__(None, None, None)
    ps2 = tc.tile_pool(name="ps2", bufs=1, space="PSUM")
    ps2_pool = ps2.__enter__()
    ph2 = [ps2_pool.tile([128, OWN_TOK], F32, space="PSUM", tag=f"h2p{m2}", name=f"h2p{m2}")
           for m2 in range(8)]
    for kc in range(32):
        w2t = wpool.tile([128, 1024], BF16, tag="w2t")
        nc.sync.dma_start(out=w2t[:], in_=t["w2"][128 * kc:128 * (kc + 1), :])
        for m2 in range(8):
            nc.tensor.matmul(ph2[m2][:], w2t[:, 128 * m2:128 * (m2 + 1)],
                             h1T[kc][:], start=(kc == 0), stop=(kc == 31))
    h2T = [poolL_p.tile([128, OWN_TOK], F32, tag=f"h2T{m2}", name=f"h2T{m2}") for m2 in range(8)]
    for m2 in range(8):
        nc.vector.tensor_scalar(h2T[m2][:], ph2[m2][:], b2_sb[:, m2:m2 + 1],
                                None, op0=OP.add)
    ps2.__exit__(None, None, None)
    ps3 = tc.tile_pool(name="ps3", bufs=2, space="PSUM")
    ps3_pool = ps3.__enter__()

    # ---------------- phase J: h transpose, add, LN, scatter ----------------
    ones_t = spool.tile([128, 1], F32, tag="ones_mk")
    nc.vector.memset(ones_t[:], 1.0)
    for ti in range(4):
        pht = ps3_pool.tile([128, 1024], F32, space="PSUM", tag="jt")
        for dd in range(8):
            nc.tensor.transpose(pht[:, 128 * dd:128 * (dd + 1)],
                                h2T[dd][:, 128 * ti:128 * (ti + 1)], ident[:])
        xgj = work.tile([128, 1024], F32, tag="xgj", bufs=1)
        nc.gpsimd.indirect_dma_start(
            out=xgj[:], out_offset=None, in_=x[:, :],
            in_offset=bass.IndirectOffsetOnAxis(ap=sel_i[:, ti:ti + 1], axis=0))
        xu = work.tile([128, 1024], F32, tag="xu", bufs=1)
        nc.vector.tensor_add(xu[:], pht[:], xgj[:])
        yt = layer_norm(xu[:])
        nc.gpsimd.indirect_dma_start(
            out=y[:, :],
            out_offset=bass.IndirectOffsetOnAxis(ap=sel_i[:, ti:ti + 1], axis=0),
            in_=yt[:], in_offset=None)
        nc.gpsimd.indirect_dma_start(
            out=t["mk_out"][:, :],
            out_offset=bass.IndirectOffsetOnAxis(ap=sel_i[:, ti:ti + 1], axis=0),
            in_=ones_t[:], in_offset=None)

    # scores output (+ b_sample)
    sco = spool.tile([128, NT_OWN], F32, tag="sco")
    nc.vector.tensor_scalar(sco[:], scores_sb[:, 0:NT_OWN], bs_rep[:, 0:1],
                            None, op0=OP.add)
    nc.sync.dma_start(
        out=t["sc_out"].ap().rearrange("(a p) o -> p (a o)", p=128),
        in_=sco[:])
    ps3.__exit__(None, None, None)
    poolL.__exit__(None, None, None)
    es.close()


# ---------------------------------------------------------------- host side
_CACHE = {}


def _get_nc():
    if "nc" not in _CACHE:
        nc = bacc.Bacc("TRN2", target_bir_lowering=False, debug=False,
                       enable_asserts=False, num_devices=N_CORES)
        build_kernel(nc)
        nc.compile()
        _CACHE["nc"] = nc
    return _CACHE["nc"]


def make_in_maps(inputs):
    bf = lambda a: np.asarray(a, np.float32).astype(ml_dtypes.bfloat16)
    f32 = lambda a: np.ascontiguousarray(np.asarray(a, np.float32))
    x = f32(inputs["x"])
    shared = {
        "wq": bf(inputs["wq"]), "wk": bf(inputs["wk"]), "wv": bf(inputs["wv"]),
        "wo": bf(inputs["w_out"]), "w1": bf(inputs["w1"]), "w2": bf(inputs["w2"]),
        "wsamp": f32(inputs["w_sample"]).reshape(D, 1),
        "bsamp": f32(inputs["b_sample"]).reshape(1, 1),
        "betas": f32(inputs["betas"]).reshape(H * DV, 1),
        "b1": f32(inputs["b1"]).reshape(DH, 1),
        "b2": f32(inputs["b2"]).reshape(D, 1),
        "gamma": f32(inputs["ln_gamma"]).reshape(D, 1),
        "lnbeta": f32(inputs["ln_beta"]).reshape(D, 1),
    }
    wpre0 = np.array([[0, 0, 0, 0, 1, 0, 0, 0]], np.float32)      # own = g0,g1
    wpre1 = np.array([[0, 0, 1, 1, 1, 0, 1, 1]], np.float32)      # own = g2,g3
    in_maps = []
    for c in range(N_CORES):
        b, hf = c // 2, c % 2
        xl = np.concatenate(
            [x[b, hf * HALF:(hf + 1) * HALF], x[b, (1 - hf) * HALF:(2 - hf) * HALF]],
            axis=0)
        m = dict(shared)
        m["x"] = np.ascontiguousarray(xl)
        m["wpre"] = wpre1 if hf else wpre0
        in_maps.append(m)
    return in_maps


def assemble(results):
    y = np.empty((B, S, D), np.float32)
    mask = np.empty((B * S, 1), np.float32)
    scores = np.empty((B * S, 1), np.float32)
    for c, r in enumerate(results):
        b, hf = c // 2, c % 2
        rows = slice(hf * HALF, (hf + 1) * HALF)
        y[b, rows] = r["y"]
        grows = slice(b * S + hf * HALF, b * S + (hf + 1) * HALF)
        mask[grows] = r["mk_out"]
        scores[grows] = r["sc_out"]
    return y, mask, scores


def kernel(**inputs):
    nc = _get_nc()
    in_maps = make_in_maps(inputs)
    res = bass_utils.run_bass_kernel_spmd(nc, in_maps, core_ids=list(range(N_CORES)))
    return assemble(res.results)
